# revision 1
# baseline (speedup 1.0000x reference)
"""GroupARouter MoE-routing kernel for 8 Trainium2 NeuronCores.

Strategy: data-parallel over batch B=8 (one batch per core). Host preps
per-core transposed gate input [tokens.T; xyz.T; ones] so the PE never
transposes the 16 MiB token matrix. Device computes the gate MLP in fp32
on PE, spatial distances via a K=4 matmul + Newton-refined sqrt, the
global dists-mean via a 1-scalar AllReduce, the exact per-expert top-k
(k=4096) membership threshold via lo/hi bisection with fused DVE count
passes, and the floor/cap/combine epilogue in an expert-major layout.
"""
import numpy as np

B, N, D, E, TOP_K = 8, 16384, 256, 8, 2
H = D // 2                      # 128
K_SEL = N * TOP_K // E          # 4096
FLOOR = min(0.05, 0.15 / 4)     # 0.0375
ALPHA = FLOOR * E               # 0.3
CAP_LOW, CAP_HIGH, T_MAX = 0.5, 0.6, 1000
NCORES = 8
CH = 512                        # tokens per phase-1 chunk
NCHUNK = N // CH                # 32
CQ = N // 128                   # 128 column-chunks of 128 tokens
NJ = CQ // 16                   # 8 transpose blocks
BISECT_ITERS = 28
SAR_RANGE = 16.0                # logits guaranteed within [-16, 16]

_CACHE = {}


def _build():
    import concourse.bacc as bacc
    import concourse.mybir as mybir
    import concourse.tile as tile

    F32 = mybir.dt.float32
    I32 = mybir.dt.int32
    AF = mybir.ActivationFunctionType
    ALU = mybir.AluOpType

    nc = bacc.Bacc("TRN2", target_bir_lowering=False, debug=False,
                   num_devices=NCORES)

    # ---- DRAM I/O
    d_gT = nc.dram_tensor("gT", (D + 4, N), F32, kind="ExternalInput")
    d_XX = nc.dram_tensor("XX", (128, CQ), F32, kind="ExternalInput")
    d_tb = nc.dram_tensor("tb", (1, 1), I32, kind="ExternalInput")
    d_W1b = nc.dram_tensor("W1b", (D + 4, H), F32, kind="ExternalInput")
    d_W2 = nc.dram_tensor("W2", (H, E), F32, kind="ExternalInput")
    d_CM = nc.dram_tensor("CM", (4, E), F32, kind="ExternalInput")
    d_B2 = nc.dram_tensor("B2", (128, E), F32, kind="ExternalInput")
    d_Bbc = nc.dram_tensor("Bbc", (128, 128), F32, kind="ExternalInput")
    d_Btk = nc.dram_tensor("Btk", (128, 128), F32, kind="ExternalInput")
    d_id = nc.dram_tensor("ident", (128, 128), F32, kind="ExternalInput")
    o_disp = nc.dram_tensor("disp", (N, E), F32, kind="ExternalOutput")
    o_comb = nc.dram_tensor("comb", (N, E), F32, kind="ExternalOutput")
    cc_in = nc.dram_tensor("cc_in", (1, 128), F32, kind="Internal")
    cc_out = nc.dram_tensor("cc_out", (1, 128), F32, kind="Internal",
                            addr_space="Shared")

    with tile.TileContext(nc) as tc:
        with (
            tc.tile_pool(name="consts", bufs=1) as cpool,
            tc.tile_pool(name="stream", bufs=3) as spool,
            tc.tile_pool(name="store", bufs=1) as tpool,
            tc.tile_pool(name="small", bufs=1) as mpool,
            tc.tile_pool(name="ph3", bufs=1) as xpool,
        ):
            # ---- constants into SBUF
            t_w0 = cpool.tile([128, H], F32)
            t_w1 = cpool.tile([128, H], F32)
            t_w2k = cpool.tile([4, H], F32)
            t_W2 = cpool.tile([H, E], F32)
            t_CM = cpool.tile([4, E], F32)
            t_B2 = cpool.tile([128, E], F32)
            t_Bbc = cpool.tile([128, 128], F32)
            t_Btk = cpool.tile([128, 128], F32)
            t_id = cpool.tile([128, 128], F32)
            t_XX = cpool.tile([128, CQ], F32)
            nc.sync.dma_start(t_w0[:], d_W1b[0:128, :])
            nc.sync.dma_start(t_w1[:], d_W1b[128:256, :])
            nc.sync.dma_start(t_w2k[:], d_W1b[256:260, :])
            nc.sync.dma_start(t_W2[:], d_W2[:, :])
            nc.sync.dma_start(t_CM[:], d_CM[:, :])
            nc.sync.dma_start(t_B2[:], d_B2[:, :])
            nc.sync.dma_start(t_Bbc[:], d_Bbc[:, :])
            nc.sync.dma_start(t_Btk[:], d_Btk[:, :])
            nc.sync.dma_start(t_id[:], d_id[:, :])
            nc.sync.dma_start(t_XX[:], d_XX[:, :])

            t_eps8 = cpool.tile([128, E], F32)
            nc.vector.memset(t_eps8[:], 1e-12)
            t_zeroEM = cpool.tile([128, NJ * 128], F32)
            nc.vector.memset(t_zeroEM[:], 0.0)
            t_onesEM = cpool.tile([128, NJ * 128], F32)
            nc.vector.memset(t_onesEM[:], 1.0)

            # cap = 0.5 + 1.1e-3 * t_b, broadcast to all partitions
            t_ti = mpool.tile([1, 1], I32)
            nc.sync.dma_start(t_ti[:], d_tb[:, :])
            t_tf = mpool.tile([1, 1], F32)
            nc.vector.tensor_copy(t_tf[:], t_ti[:])
            t_cap1 = mpool.tile([1, 1], F32)
            nc.vector.tensor_scalar(
                t_cap1[:], t_tf[:], (CAP_HIGH + CAP_LOW) / T_MAX, CAP_LOW,
                op0=ALU.mult, op1=ALU.add)
            t_cap = mpool.tile([128, 1], F32)
            nc.gpsimd.partition_broadcast(t_cap[:], t_cap1[:])

            # ---- token-major accumulators
            TLc = tpool.tile([128, CQ, E], F32)    # content logits
            TLd = tpool.tile([128, CQ, E], F32)    # dist^2 then dist

            # ---- phase 1: stream 32 chunks of 512 tokens
            with (
                tc.tile_pool(name="ps_h", bufs=2, space="PSUM") as ps_h,
                tc.tile_pool(name="ps_l", bufs=2, space="PSUM") as ps_l,
                tc.tile_pool(name="ps_d", bufs=2, space="PSUM") as ps_d,
            ):
                for c in range(NCHUNK):
                    sl = slice(c * CH, (c + 1) * CH)
                    t_g = spool.tile([128, 2, CH], F32, tag="gchunk")
                    nc.sync.dma_start(t_g[:, 0, :], d_gT[0:128, sl])
                    nc.sync.dma_start(t_g[:, 1, :], d_gT[128:256, sl])
                    t_x1 = spool.tile([4, CH], F32, tag="xchunk")
                    nc.sync.dma_start(t_x1[:], d_gT[256:260, sl])

                    p_h = ps_h.tile([H, CH], F32)
                    nc.tensor.matmul(p_h[:], t_w0[:], t_g[:, 0, :],
                                     start=True, stop=False)
                    nc.tensor.matmul(p_h[:], t_w1[:], t_g[:, 1, :],
                                     start=False, stop=False)
                    nc.tensor.matmul(p_h[:], t_w2k[:], t_x1[:],
                                     start=False, stop=True)
                    t_hg = spool.tile([H, CH], F32, tag="hg")
                    nc.scalar.activation(t_hg[:], p_h[:], AF.Gelu)

                    p_L = ps_l.tile([128, 4, E], F32)
                    p_D = ps_d.tile([128, 4, E], F32)
                    for q in range(4):
                        qs = slice(q * 128, (q + 1) * 128)
                        nc.tensor.matmul(p_L[:, q, :], t_hg[:, qs], t_W2[:],
                                         start=True, stop=True)
                        nc.tensor.matmul(p_D[:, q, :], t_x1[:, qs], t_CM[:],
                                         start=True, stop=True)
                    nc.vector.tensor_copy(TLc[:, c * 4:(c + 1) * 4, :], p_L[:])
                    for q in range(4):
                        cq = c * 4 + q
                        # dist^2 = max(xx + (cc - 2 x.c), eps)
                        nc.vector.scalar_tensor_tensor(
                            TLd[:, cq, :], p_D[:, q, :], t_XX[:, cq:cq + 1],
                            t_eps8[:], op0=ALU.add, op1=ALU.max)

            # ---- phase 1.5: bulk sqrt + Newton, dist-sum, AllReduce, logits
            TLd_f = TLd[:].rearrange("p a b -> p (a b)")
            TLc_f = TLc[:].rearrange("p a b -> p (a b)")
            with tc.tile_pool(name="ps_m", bufs=2, space="PSUM") as ps_m:
                t_s0 = tpool.tile([128, CQ * E], F32)
                nc.scalar.activation(t_s0[:], TLd_f, AF.Sqrt)
                t_r0 = tpool.tile([128, CQ * E], F32)
                nc.vector.reciprocal(t_r0[:], t_s0[:])
                nc.vector.tensor_tensor(t_r0[:], TLd_f, t_r0[:], op=ALU.mult)
                nc.vector.tensor_tensor(t_r0[:], t_r0[:], t_s0[:], op=ALU.add)
                nc.vector.tensor_scalar(TLd_f, t_r0[:], 0.5, None,
                                        op0=ALU.mult)

                t_dsum = mpool.tile([128, 1], F32)
                nc.vector.tensor_reduce(t_dsum[:], TLd[:],
                                        axis=mybir.AxisListType.XY, op=ALU.add)
                p_tot = ps_m.tile([128, 1], F32)
                nc.tensor.matmul(p_tot[:], t_onesEM[:, 0:128], t_dsum[:],
                                 start=True, stop=True)
                t_S1 = mpool.tile([1, 1], F32)
                nc.scalar.copy(t_S1[:], p_tot[0:1, :])
                t_S = mpool.tile([1, 128], F32)
                nc.vector.tensor_copy(t_S[:], t_S1[:].broadcast_to((1, 128)))
                nc.sync.dma_start(cc_in[:, :], t_S[:])
                nc.gpsimd.collective_compute(
                    "AllReduce", ALU.add, ins=[cc_in[:, :]],
                    outs=[cc_out[:, :]], replica_groups=[list(range(NCORES))])
                t_Sall = mpool.tile([1, 1], F32)
                nc.sync.dma_start(t_Sall[:], cc_out[:, 0:1])
                # beta = -1 / (S/(B*N*E) + 1e-6); Newton-refined reciprocal
                t_m = mpool.tile([1, 1], F32)
                nc.vector.tensor_scalar(t_m[:], t_Sall[:], 1.0 / (B * N * E),
                                        1e-6, op0=ALU.mult, op1=ALU.add)
                t_rm = mpool.tile([1, 1], F32)
                nc.vector.reciprocal(t_rm[:], t_m[:])
                t_mr = mpool.tile([1, 1], F32)
                nc.vector.tensor_tensor(t_mr[:], t_m[:], t_rm[:], op=ALU.mult)
                nc.vector.tensor_scalar(t_mr[:], t_mr[:], -1.0, 2.0,
                                        op0=ALU.mult, op1=ALU.add)
                t_beta1 = mpool.tile([1, 1], F32)
                nc.vector.tensor_tensor(t_beta1[:], t_rm[:], t_mr[:],
                                        op=ALU.mult)
                nc.vector.tensor_scalar(t_beta1[:], t_beta1[:], -1.0, None,
                                        op0=ALU.mult)
                t_beta = mpool.tile([128, 1], F32)
                nc.gpsimd.partition_broadcast(t_beta[:], t_beta1[:])

                # logits = beta*dist + content + b2
                TL = tpool.tile([128, CQ, E], F32)
                TL_f = TL[:].rearrange("p a b -> p (a b)")
                nc.vector.scalar_tensor_tensor(TL_f, TLd_f, t_beta[:], TLc_f,
                                               op0=ALU.mult, op1=ALU.add)
                nc.vector.tensor_tensor(
                    TL[:], TL[:],
                    t_B2[:].unsqueeze(1).broadcast_to((128, CQ, E)),
                    op=ALU.add)

            # ---- phase 2: expert-major transpose + lo/hi bisection
            EM = tpool.tile([128, NJ, 128], F32)
            with (
                tc.tile_pool(name="ps_t", bufs=2, space="PSUM") as ps_t,
                tc.tile_pool(name="ps_c", bufs=2, space="PSUM") as ps_c,
            ):
                for j in range(NJ):
                    p_T = ps_t.tile([128, 128], F32)
                    nc.tensor.transpose(
                        p_T[:], TL[:, j * 16:(j + 1) * 16, :].rearrange(
                            "p a b -> p (a b)"), t_id[:])
                    nc.scalar.copy(EM[:, j, :], p_T[:])
                EM_f = EM[:].rearrange("p a b -> p (a b)")

                t_lo = mpool.tile([128, 1], F32)
                t_hi = mpool.tile([128, 1], F32)
                nc.vector.memset(t_lo[:], -SAR_RANGE)
                nc.vector.memset(t_hi[:], SAR_RANGE)
                t_mid = mpool.tile([128, 1], F32)
                t_cnt = mpool.tile([128, 1], F32)
                t_ge = mpool.tile([128, 1], F32)
                t_gei = mpool.tile([128, 1], F32)
                t_dl = mpool.tile([128, 1], F32)
                t_dh = mpool.tile([128, 1], F32)
                t_junk = tpool.tile([128, NJ * 128], F32)
                for it in range(BISECT_ITERS):
                    nc.vector.tensor_tensor(t_mid[:], t_lo[:], t_hi[:],
                                            op=ALU.add)
                    nc.vector.tensor_scalar(t_mid[:], t_mid[:], 0.5, None,
                                            op0=ALU.mult)
                    nc.vector.scalar_tensor_tensor(
                        t_junk[:], EM_f, t_mid[:], t_onesEM[:],
                        op0=ALU.is_ge, op1=ALU.mult, accum_out=t_cnt[:])
                    p_ct = ps_c.tile([128, 1], F32)
                    nc.tensor.matmul(p_ct[:], t_Bbc[:], t_cnt[:],
                                     start=True, stop=True)
                    # ge = 1 if count(>=mid) >= k  (mid still <= v_k: lo=mid)
                    nc.vector.tensor_scalar(t_ge[:], p_ct[:], float(K_SEL),
                                            None, op0=ALU.is_ge)
                    nc.vector.tensor_scalar(t_gei[:], t_ge[:], -1.0, 1.0,
                                            op0=ALU.mult, op1=ALU.add)
                    nc.vector.scalar_tensor_tensor(
                        t_dl[:], t_mid[:], t_lo[:], t_ge[:],
                        op0=ALU.subtract, op1=ALU.mult)
                    nc.vector.scalar_tensor_tensor(
                        t_dh[:], t_mid[:], t_hi[:], t_gei[:],
                        op0=ALU.subtract, op1=ALU.mult)
                    nc.vector.tensor_tensor(t_lo[:], t_lo[:], t_dl[:],
                                            op=ALU.add)
                    nc.vector.tensor_tensor(t_hi[:], t_hi[:], t_dh[:],
                                            op=ALU.add)

                # ---- phase 3 (expert-major, per-partition scalars)
                t_sig = xpool.tile([128, NJ * 128], F32)
                nc.scalar.activation(t_sig[:], EM_f, AF.Sigmoid)
                t_d1 = xpool.tile([128, NJ * 128], F32)
                nc.vector.scalar_tensor_tensor(t_d1[:], EM_f, t_lo[:],
                                               t_sig[:], op0=ALU.is_ge,
                                               op1=ALU.mult)
                nc.vector.tensor_scalar(t_d1[:], t_d1[:], 1.0 - ALPHA, FLOOR,
                                        op0=ALU.mult, op1=ALU.add)
                t_exc = xpool.tile([128, NJ * 128], F32)
                nc.vector.scalar_tensor_tensor(t_exc[:], t_d1[:], t_cap[:],
                                               t_zeroEM[:], op0=ALU.subtract,
                                               op1=ALU.max)
                t_capd = xpool.tile([128, NJ * 128], F32)
                nc.vector.tensor_tensor(t_capd[:], t_d1[:], t_exc[:],
                                        op=ALU.subtract)
                t_mh = xpool.tile([128, NJ * 128], F32)
                nc.vector.scalar_tensor_tensor(t_mh[:], t_capd[:], t_cap[:],
                                               t_zeroEM[:], op0=ALU.subtract,
                                               op1=ALU.min)

            with (
                tc.tile_pool(name="ps_e", bufs=1, space="PSUM") as ps_e,
                tc.tile_pool(name="ps_s", bufs=1, space="PSUM") as ps_s,
                tc.tile_pool(name="ps_o", bufs=2, space="PSUM") as ps_o,
            ):
                p_es = ps_e.tile([128, NJ * 128], F32, tag="es")
                p_hs = ps_e.tile([128, NJ * 128], F32, tag="hss")
                for half in range(2):
                    hsl = slice(half * 512, (half + 1) * 512)
                    nc.tensor.matmul(p_es[:, hsl], t_Btk[:], t_exc[:, hsl],
                                     start=True, stop=True)
                    nc.tensor.matmul(p_hs[:, hsl], t_Btk[:], t_mh[:, hsl],
                                     start=True, stop=True)
                t_hsum = xpool.tile([128, NJ * 128], F32)
                nc.vector.tensor_scalar(t_hsum[:], p_hs[:], -1.0, 1e-8,
                                        op0=ALU.mult, op1=ALU.max)
                t_hr = xpool.tile([128, NJ * 128], F32)
                nc.vector.reciprocal(t_hr[:], t_hsum[:])
                t_ratio = xpool.tile([128, NJ * 128], F32)
                nc.vector.scalar_tensor_tensor(t_ratio[:], t_mh[:], -1.0,
                                               t_hr[:], op0=ALU.mult,
                                               op1=ALU.mult)
                t_corr = xpool.tile([128, NJ * 128], F32)
                nc.vector.tensor_tensor(t_corr[:], t_ratio[:], p_es[:],
                                        op=ALU.mult)
                t_disp = xpool.tile([128, NJ * 128], F32)
                nc.vector.tensor_tensor(t_disp[:], t_capd[:], t_corr[:],
                                        op=ALU.add)
                p_ds = ps_s.tile([128, NJ * 128], F32)
                for half in range(2):
                    hsl = slice(half * 512, (half + 1) * 512)
                    nc.tensor.matmul(p_ds[:, hsl], t_Btk[:], t_disp[:, hsl],
                                     start=True, stop=True)
                t_den = xpool.tile([128, NJ * 128], F32)
                nc.vector.tensor_scalar(t_den[:], p_ds[:], 1e-8, None,
                                        op0=ALU.add)
                t_dr = xpool.tile([128, NJ * 128], F32)
                nc.vector.reciprocal(t_dr[:], t_den[:])
                t_comb = xpool.tile([128, NJ * 128], F32)
                nc.vector.tensor_tensor(t_comb[:], t_disp[:], t_dr[:],
                                        op=ALU.mult)

                # ---- outputs: transpose back to token-major, DMA contiguous
                disp_v = t_disp[:].rearrange("p (a b) -> p a b", a=NJ)
                comb_v = t_comb[:].rearrange("p (a b) -> p a b", a=NJ)
                for j in range(NJ):
                    rows = slice(j * 16 * 128, (j + 1) * 16 * 128)
                    for src, dst in ((disp_v, o_disp), (comb_v, o_comb)):
                        p_O = ps_o.tile([128, 128], F32, tag="otr")
                        nc.tensor.transpose(p_O[:], src[:, j, :], t_id[:])
                        t_O = spool.tile([128, 128], F32, tag="osb")
                        nc.scalar.copy(t_O[:], p_O[:])
                        nc.sync.dma_start(
                            dst[rows, :].rearrange("(g tl) e -> tl g e", g=16),
                            t_O[:].rearrange("tl (g e) -> tl g e", g=16))

    nc.compile()
    return nc


def _consts():
    ident = np.eye(128, dtype=np.float32)
    pe = np.arange(128)
    Bbc = (pe[:, None] % 8 == pe[None, :] % 8).astype(np.float32)
    Btk = (pe[:, None] // 8 == pe[None, :] // 8).astype(np.float32)
    return ident, Bbc, Btk


def kernel(tokens, spatial_xyz, W1, b1, W2, b2, centers, t):
    tokens = np.ascontiguousarray(np.asarray(tokens, np.float32))
    xyz = np.ascontiguousarray(np.asarray(spatial_xyz, np.float32))
    W1 = np.asarray(W1, np.float32)
    b1 = np.asarray(b1, np.float32)
    W2 = np.asarray(W2, np.float32)
    b2 = np.asarray(b2, np.float32)
    centers = np.asarray(centers, np.float32)
    t = np.asarray(t).astype(np.int32)

    from concourse import bass_utils
    if "nc" not in _CACHE:
        _CACHE["nc"] = _build()
    nc = _CACHE["nc"]

    ident, Bbc, Btk = _consts()
    W1b = np.zeros((D + 4, H), np.float32)
    W1b[:D + 3] = W1
    W1b[D + 3] = b1
    CM = np.zeros((4, E), np.float32)
    CM[0:3] = -2.0 * centers.T
    CM[3] = (centers.astype(np.float64) ** 2).sum(-1).astype(np.float32)
    B2 = np.tile(b2[None, :], (128, 1)).astype(np.float32)
    W2c = np.ascontiguousarray(W2)

    in_maps = []
    for b in range(B):
        gT = np.empty((D + 4, N), np.float32)
        gT[0:D] = tokens[b].T
        gT[D:D + 3] = xyz[b].T
        gT[D + 3] = 1.0
        xx = (xyz[b].astype(np.float64) ** 2).sum(-1).astype(np.float32)
        XX = np.ascontiguousarray(xx.reshape(CQ, 128).T)
        in_maps.append(dict(
            gT=gT, XX=XX, tb=np.array([[t[b]]], np.int32), W1b=W1b, W2=W2c,
            CM=CM, B2=B2, Bbc=Bbc, Btk=Btk, ident=ident))

    import os
    trace = os.environ.get("KERNEL_TRACE", "0") == "1"
    res = bass_utils.run_bass_kernel_spmd(nc, in_maps, list(range(NCORES)),
                                          trace=trace)
    _CACHE["exec_time_ns"] = getattr(res, "exec_time_ns", None)
    _CACHE["last_res"] = res
    disp = np.stack([r["disp"] for r in res.results])
    comb = np.stack([r["comb"] for r in res.results])
    return disp, comb


if __name__ == "__main__":
    rng = np.random.default_rng(0)
    ins = dict(
        tokens=rng.standard_normal((B, N, D)).astype(np.float32),
        spatial_xyz=rng.standard_normal((B, N, 3)).astype(np.float32),
        W1=(rng.standard_normal((D + 3, H)) / np.sqrt(D + 3)).astype(np.float32),
        b1=np.zeros(H, np.float32),
        W2=(rng.standard_normal((H, E)) / np.sqrt(H)).astype(np.float32),
        b2=np.zeros(E, np.float32),
        centers=(rng.standard_normal((E, 3)) * 10).astype(np.float32),
        t=rng.integers(0, T_MAX, B).astype(np.int32),
    )
    d, c = kernel(**ins)
    print("disp", d.shape, d.dtype, "comb", c.shape, c.dtype)

